# revision 1
# baseline (speedup 1.0000x reference)
"""Trainium2 Bass kernel for nn_FFFFanout (moe_routing tree-MLP).

Contract: kernel(**inputs) takes FULL unsharded numpy inputs
  oldx  [2, 2048, 1024] f32
  W_in  [21840, 1024]   f32
  b_in  [21840]         f32
  W_out [1024, 21840]   f32
returns [2, 2048, 1024] f32.

Strategy: data-parallel over the 4096 flattened tokens -> 512 per core on 8
NeuronCores. Device layout is "f-major": dev_hid(f, p, g) = f*5632 + p*1408 + g
with G padded 1365 -> 1408 so each (p, g)-plane tile aligns to 128 partitions.
This makes the group-of-4 argmax an elementwise max across 4 f-plane tiles,
the tree mask multiply partition-aligned, and both matmuls contraction-friendly
(activations live as [hid, tokens]).

Per core:
  phase A: logits.T tiles [128h, 512tok] = W_inT.T @ x.T (PSUM, K=1024 in 8
           chunks), gelu+bias on ACT, spill act tiles to DRAM. Routing region
           (g < 341, i.e. tree levels 0..4) in fp32, rest in fp32r.
           Group argmax index dec in {0..3} from DVE compares (first-max
           tie-break identical to np.argmax).
  phase B: gather-free tree cascade: child(g, m) = 4g+1+m, so
           sel[d+1][4j+f] = sel[d][j] * (dec[j] == f), levels written into a
           [5632, tok] mask via partition-interleave SBUF DMAs.
  phase C: masked = act * mask (DVE, out fp32r), out.T [1024d, 512tok]
           accumulated over all 176 h-tiles in all 8 PSUM banks, fp32r matmuls.
"""
import sys

if "/opt/trn_rl_repo" not in sys.path:
    sys.path.insert(0, "/opt/trn_rl_repo")

from contextlib import ExitStack

import numpy as np

import concourse.bass as bass  # noqa: F401
import concourse.mybir as mybir
import concourse.tile as tile
from concourse import bacc
from concourse.bass_utils import run_bass_kernel_spmd

F32 = mybir.dt.float32
F32R = mybir.dt.float32r
BF16 = mybir.dt.bfloat16

D = 1024
P = 4
DEPTH = 5
FAN = 4
G = 1365
HID = 21840
Gp = 1408            # 11 * 128
Sp = P * Gp          # 5632  (one f-plane)
HIDp = FAN * Sp      # 22528
NT = HIDp // 128     # 176 h-tiles
NPG = Sp // 128      # 44 (p,g) tiles per f-plane
T = 512              # tokens per core
NCORES = 8
KC = D // 128        # 8 contraction chunks

DEBUG_DUMPS = False

ROUTE_SUBS = 3                  # g < 341 lives in subtiles 0..2 of each p
ROUTE_G = 341                   # groups 0..340 are tree levels 0..4

# routing h-tiles ordered (p, sub, f): the four f-planes of one (p,g)-tile are
# adjacent because the group argmax consumes all four together
ROUTE_TILES = [f * NPG + p * 11 + sub
               for p in range(P) for sub in range(ROUTE_SUBS) for f in range(FAN)]
ROUTE_SET = set(ROUTE_TILES)
NONROUTE_TILES = [t for t in range(NT) if t not in ROUTE_SET]


def _interleave():
    """Phase A order: routing quads (PE-heavy fp32, little DMA) interleaved
    with non-routing tiles (DMA-heavy f32r) so neither resource starves."""
    order = []
    nr = list(NONROUTE_TILES)
    chunk = -(-len(nr) // (len(ROUTE_TILES) // FAN))     # ~11 per quad
    for qi in range(len(ROUTE_TILES) // FAN):
        order.extend(ROUTE_TILES[qi * FAN:(qi + 1) * FAN])
        order.extend(nr[qi * chunk:(qi + 1) * chunk])
    order.extend(nr[(len(ROUTE_TILES) // FAN) * chunk:])
    assert sorted(order) == list(range(NT))
    return order


# phase A / phase C production+consumption order
TILE_ORDER = ROUTE_TILES + NONROUTE_TILES


def _adj_groups(order, maxlen=2):
    """Group runs of memory-adjacent entries (up to maxlen) for batched DMA."""
    groups = []
    i = 0
    while i < len(order):
        j = i + 1
        while (j < len(order) and j - i < maxlen
               and order[j] == order[j - 1] + 1):
            j += 1
        groups.append(tuple(order[i:j]))
        i = j
    return groups


NR_GROUPS = _adj_groups(NONROUTE_TILES)
C_GROUPS = _adj_groups(TILE_ORDER)


def _platform(d):
    return (FAN ** d - 1) // 3


def _segments(q0, q1, *bases):
    """Split [q0, q1) at every multiple of 128 relative to each base offset."""
    cuts = {q0, q1}
    for b in bases:
        k = b + ((q0 - b) // 128 + 1) * 128
        while k < q1:
            cuts.add(k)
            k += 128
    cs = sorted(cuts)
    return list(zip(cs[:-1], cs[1:]))


def build_nc():
    nc = bacc.Bacc("TRN2", target_bir_lowering=False, debug=False,
                   num_devices=NCORES)

    xT32 = nc.dram_tensor("xT32", [D, T], F32, kind="ExternalInput").ap()
    xTr = nc.dram_tensor("xTr", [D, T], F32R, kind="ExternalInput").ap()
    Wroute = nc.dram_tensor("Wroute", [D, len(ROUTE_TILES) * 128], F32,
                            kind="ExternalInput").ap()
    Wfull = nc.dram_tensor("Wfull", [D, HIDp], F32R, kind="ExternalInput").ap()
    bvec = nc.dram_tensor("bvec", [128, NT], F32, kind="ExternalInput").ap()
    WoT = nc.dram_tensor("WoT", [HIDp, D], F32R, kind="ExternalInput").ap()
    outT = nc.dram_tensor("outT", [D, T], F32, kind="ExternalOutput").ap()
    if DEBUG_DUMPS:
        decdbg = nc.dram_tensor("decdbg", [128, P * ROUTE_SUBS, T], F32,
                                kind="ExternalOutput").ap()
        maskdbg = nc.dram_tensor("maskdbg", [128, NPG, T], BF16,
                                 kind="ExternalOutput").ap()

    with tile.TileContext(nc) as tc, ExitStack() as top:
        perm = top.enter_context(tc.tile_pool(name="perm", bufs=1))
        dramp = top.enter_context(tc.tile_pool(name="dram", bufs=1, space="DRAM"))

        mask = perm.tile([128, NPG, T], BF16)        # 5.8 MB
        dec = perm.tile([128, P * ROUTE_SUBS, T], F32)   # 3.1 MB
        bt = perm.tile([128, NT], F32)
        nc.gpsimd.dma_start(bt[:], bvec[:])
        nc.gpsimd.memset(mask[:], 0.0)

        act_scr = [dramp.tile([128, T], F32, tag=f"as{t}", name=f"as{t}")
                   for t in range(NT)]

        # ---------------- phase A: matmul1 + gelu + dec ----------------
        with ExitStack() as pa:
            constp = pa.enter_context(tc.tile_pool(name="xconst", bufs=1))
            wrp = pa.enter_context(tc.tile_pool(name="wroute", bufs=3))
            wfp = pa.enter_context(tc.tile_pool(name="wfull", bufs=5))
            actp = pa.enter_context(tc.tile_pool(name="act", bufs=2))
            tmpp = pa.enter_context(tc.tile_pool(name="tmp", bufs=2))
            psA = pa.enter_context(tc.tile_pool(name="psA", bufs=4, space="PSUM"))

            xt32 = constp.tile([128, KC, T], F32)
            nc.sync.dma_start(xt32[:], xT32.rearrange("(c p) t -> p c t", p=128))
            xtr = constp.tile([128, KC, T], F32R)
            nc.sync.dma_start(xtr[:], xTr.rearrange("(c p) t -> p c t", p=128))

            route_acts = {}
            route_rank = {t: i for i, t in enumerate(ROUTE_TILES)}

            def a_tile(t, w_sel, x_op, tag):
                ps = psA.tile([128, T], F32, tag="ps", name=f"ps_{t}")
                for c in range(KC):
                    nc.tensor.matmul(ps[:], w_sel(c), x_op[:, c, :],
                                     start=(c == 0), stop=(c == KC - 1))
                a = actp.tile([128, T], F32, tag=tag, name=f"a_{t}")
                nc.scalar.activation(a[:], ps[:],
                                     mybir.ActivationFunctionType.Gelu,
                                     bias=bt[:, t:t + 1], scale=1.0)
                nc.sync.dma_start(act_scr[t][:], a[:])
                return a

            for t in ROUTE_TILES:
                rt = route_rank[t]
                w = wrp.tile([128, KC, 128], F32, tag="wr")
                nc.sync.dma_start(
                    w[:],
                    Wroute[:, rt * 128:(rt + 1) * 128]
                    .rearrange("(c p) h -> p c h", p=128))
                f = t // NPG
                a = a_tile(t, lambda c, w=w: w[:, c, :], xt32, f"act{f}")
                route_acts[t] = a
                rem = t % NPG
                p, sub = divmod(rem, 11)
                if f == FAN - 1:
                    # all four f-planes of (p, sub) live -> dec
                    a0 = route_acts.pop(0 * NPG + rem)
                    a1 = route_acts.pop(1 * NPG + rem)
                    a2 = route_acts.pop(2 * NPG + rem)
                    a3 = route_acts.pop(3 * NPG + rem)
                    m01 = tmpp.tile([128, T], F32, tag="m01")
                    m23 = tmpp.tile([128, T], F32, tag="m23")
                    nc.vector.tensor_tensor(m01[:], a0[:], a1[:], mybir.AluOpType.max)
                    nc.vector.tensor_tensor(m23[:], a2[:], a3[:], mybir.AluOpType.max)
                    b1t = tmpp.tile([128, T], mybir.dt.uint8, tag="b1")
                    nc.vector.tensor_tensor(b1t[:], m23[:], m01[:], mybir.AluOpType.is_gt)
                    b01 = tmpp.tile([128, T], F32, tag="b01")
                    nc.vector.tensor_tensor(b01[:], a1[:], a0[:], mybir.AluOpType.is_gt)
                    b23 = tmpp.tile([128, T], F32, tag="b23")
                    nc.vector.tensor_tensor(b23[:], a3[:], a2[:], mybir.AluOpType.is_gt)
                    b0t = tmpp.tile([128, T], F32, tag="b0")
                    nc.vector.select(b0t[:], b1t[:], b23[:], b01[:])
                    nc.vector.scalar_tensor_tensor(
                        dec[:, p * ROUTE_SUBS + sub, :], b1t[:], 2.0, b0t[:],
                        op0=mybir.AluOpType.mult, op1=mybir.AluOpType.add)

            # non-routing tiles, W loaded two memory-adjacent tiles per DMA
            for g in NR_GROUPS:
                t0g = g[0]
                w = wfp.tile([128, KC, 128 * len(g)], F32R, tag="wf")
                nc.sync.dma_start(
                    w[:],
                    Wfull[:, t0g * 128:(t0g + len(g)) * 128]
                    .rearrange("(c p) h -> p c h", p=128))
                for u, t in enumerate(g):
                    a_tile(t, lambda c, w=w, u=u: w[:, c, u * 128:(u + 1) * 128],
                           xtr, "actn")

            # ---------------- phase B: cascade ----------------
            # Engine APs need 32-aligned partition starts: product ops run on
            # 32-aligned padded windows (junk lanes never scattered out).
            # prodI rows are q-aligned (same partition as dec/mask source).
            cascp = pa.enter_context(tc.tile_pool(name="casc", bufs=1))
            prodI = cascp.tile([128, ROUTE_SUBS, FAN, T], BF16)

            for p in range(P):
                base = p * Gp
                # level 0: mask[q=base]=1; rows 1..31 get junk 1.0s that every
                # level-d scatter fully overwrites before level d+1 reads them
                nc.vector.memset(mask[0:32, p * 11, :], 1.0)
                for d in range(DEPTH):
                    plat, platn, n = _platform(d), _platform(d + 1), FAN ** d
                    q0 = base + plat
                    # products (dec[q] == f) * sel[q] -> prodI[q, f]
                    for (qa, qb) in _segments(q0, q0 + n, 0):
                        sub = (qa - base) // 128
                        pr_a, pr_b = qa % 128, (qb - 1) % 128 + 1
                        al_a = pr_a - pr_a % 32
                        al_b = min(128, -(-pr_b // 32) * 32)
                        for f in range(FAN):
                            nc.vector.scalar_tensor_tensor(
                                prodI[al_a:al_b, sub, f, :],
                                dec[al_a:al_b, p * ROUTE_SUBS + sub, :],
                                float(f),
                                mask[al_a:al_b, p * 11 + sub, :],
                                op0=mybir.AluOpType.is_equal,
                                op1=mybir.AluOpType.mult)
                    # scatter prodI -> mask at q' = base + platn + 4j + f
                    c0 = base + platn
                    for r in range(c0 // 128, (c0 + 4 * n - 1) // 128 + 1):
                        lo, hi = max(c0, 128 * r), min(c0 + 4 * n, 128 * (r + 1))
                        for f in range(FAN):
                            ja = max(0, -(-(lo - c0 - f) // 4))
                            jb = min(n, (hi - 1 - c0 - f) // 4 + 1)
                            if ja >= jb:
                                continue
                            for (j1, j2) in _segments(ja, jb, -q0):
                                qsrc = q0 + j1
                                sub = (qsrc - base) // 128
                                pd = (c0 + 4 * j1 + f) % 128
                                nc.sync.dma_start(
                                    mask[pd:pd + 4 * (j2 - j1 - 1) + 1:4, r, :],
                                    prodI[qsrc % 128:qsrc % 128 + (j2 - j1),
                                          sub, f, :])

        # ---------------- phase C: mask-mult + matmul2 ----------------
        with ExitStack() as pc:
            wop = pc.enter_context(tc.tile_pool(name="wo", bufs=4))
            actc = pc.enter_context(tc.tile_pool(name="actc", bufs=6))
            mskp = pc.enter_context(tc.tile_pool(name="msk", bufs=6))
            outp = pc.enter_context(tc.tile_pool(name="outp", bufs=1))
            psC = pc.enter_context(tc.tile_pool(name="psC", bufs=1, space="PSUM"))

            cps = psC.tile([128, KC, T], F32)
            i = 0
            for g in C_GROUPS:
                t0g = g[0]
                wo = wop.tile([128, len(g), D], F32R, tag="wo")
                nc.sync.dma_start(
                    wo[:],
                    WoT[t0g * 128:(t0g + len(g)) * 128, :]
                    .rearrange("(u p) d -> p u d", p=128))
                for u, t in enumerate(g):
                    a = actc.tile([128, T], F32, tag="a")
                    nc.sync.dma_start(a[:], act_scr[t][:])
                    m = mskp.tile([128, T], F32R, tag="m")
                    nc.vector.tensor_tensor(m[:], a[:], mask[:, t % NPG, :],
                                            mybir.AluOpType.mult)
                    for dd in range(KC):
                        nc.tensor.matmul(cps[:, dd, :],
                                         wo[:, u, dd * 128:(dd + 1) * 128], m[:],
                                         start=(i == 0), stop=(i == NT - 1))
                    i += 1

            if DEBUG_DUMPS:
                nc.sync.dma_start(decdbg[:], dec[:])
                nc.sync.dma_start(maskdbg[:], mask[:])
            osb = outp.tile([128, KC, T], F32)
            for dd in range(KC):
                nc.vector.tensor_copy(osb[:, dd, :], cps[:, dd, :])
            nc.sync.dma_start(outT.rearrange("(c p) t -> p c t", p=128), osb[:])

    nc.compile()
    return nc


_NC_CACHE = None


def _get_nc():
    global _NC_CACHE
    if _NC_CACHE is None:
        _NC_CACHE = build_nc()
    return _NC_CACHE


def _prep_inputs(oldx, W_in, b_in, W_out):
    x = np.ascontiguousarray(np.asarray(oldx, np.float32).reshape(-1, D))
    xT = np.ascontiguousarray(x.T)                      # [D, B]

    Wr = np.asarray(W_in, np.float32).reshape(P, G, FAN, D)
    W_dev = np.zeros((FAN, P, Gp, D), np.float32)
    W_dev[:, :, :G, :] = Wr.transpose(2, 0, 1, 3)
    W_dev = W_dev.reshape(HIDp, D)
    WT_dev = np.ascontiguousarray(W_dev.T)              # [D, HIDp]

    # routing columns, ordered like ROUTE_TILES
    cols = []
    for t in ROUTE_TILES:
        cols.append(WT_dev[:, t * 128:(t + 1) * 128])
    Wroute = np.ascontiguousarray(np.concatenate(cols, axis=1))

    br = np.asarray(b_in, np.float32).reshape(P, G, FAN)
    b_dev = np.zeros((FAN, P, Gp), np.float32)
    b_dev[:, :, :G] = br.transpose(2, 0, 1)
    b_dev = np.ascontiguousarray(b_dev.reshape(HIDp).reshape(NT, 128).T)

    Wo = np.asarray(W_out, np.float32).reshape(D, P, G, FAN)
    Wo_dev = np.zeros((FAN, P, Gp, D), np.float32)
    Wo_dev[:, :, :G, :] = Wo.transpose(3, 1, 2, 0)
    WoT_dev = np.ascontiguousarray(Wo_dev.reshape(HIDp, D))

    return xT, Wroute, WT_dev, b_dev, WoT_dev


_LAST_RES = None


def run(oldx, W_in, b_in, W_out, trace=False):
    nc = _get_nc()
    xT, Wroute, WT_dev, b_dev, WoT_dev = _prep_inputs(oldx, W_in, b_in, W_out)

    in_maps = []
    for c in range(NCORES):
        xs = np.ascontiguousarray(xT[:, c * T:(c + 1) * T])
        in_maps.append({
            "xT32": xs, "xTr": xs,
            "Wroute": Wroute, "Wfull": WT_dev,
            "bvec": b_dev, "WoT": WoT_dev,
        })
    res = run_bass_kernel_spmd(nc, in_maps, list(range(NCORES)), trace=trace)
    global _LAST_RES
    _LAST_RES = res

    outT = np.concatenate([res.results[c]["outT"] for c in range(NCORES)],
                          axis=1)                        # [D, B]
    out = np.ascontiguousarray(outT.T).reshape(np.asarray(oldx).shape)
    return out.astype(np.float32), res


def kernel(oldx, W_in, b_in, W_out):
    out, _ = run(oldx, W_in, b_in, W_out, trace=False)
    return out



# revision 2
# speedup vs baseline: 1.3059x; 1.3059x over previous
"""Trainium2 Bass kernel for nn_FFFFanout (moe_routing tree-MLP).

Contract: kernel(**inputs) takes FULL unsharded numpy inputs
  oldx  [2, 2048, 1024] f32
  W_in  [21840, 1024]   f32
  b_in  [21840]         f32
  W_out [1024, 21840]   f32
returns [2, 2048, 1024] f32.

Strategy: data-parallel over the 4096 flattened tokens -> 512 per core on 8
NeuronCores. Device layout is "f-major": dev_hid(f, p, g) = f*5632 + p*1408 + g
with G padded 1365 -> 1408 so each (p, g)-plane tile aligns to 128 partitions.
This makes the group-of-4 argmax an elementwise max across 4 f-plane tiles,
the tree mask multiply partition-aligned, and both matmuls contraction-friendly
(activations live as [hid, tokens]).

Precision plan (PE cost: fp32=4 cyc/row, fp32r/bf16=1):
  - routing decisions for tree levels 0..3 (groups 0..84, inside subtile 0 of
    each p) must match fp32 argmax exactly -> logits via 3-pass split-bf16
    (x_hi@W_hi + x_lo@W_hi + x_hi@W_lo, ~20 effective mantissa bits).
  - level-4 decisions (groups 85..340, subtiles 1..2 + tail of 0) tolerate the
    rare flip (it only swaps one leaf contribution) -> single-pass fp32r
    (measured ~12.7 bits on HW).
  - all value paths (non-routing acts, masked acts, both weight matrices) in
    bf16: same PE rate as fp32r, half the HBM traffic.

Per core:
  phase A: logits.T tiles [128h, 512tok] (PSUM, K=1024 in 8 chunks per pass),
           gelu+bias on ACT -> act tiles, spill to DRAM (f32 for route tiles,
           bf16 elsewhere). Group argmax index dec in {0..3} from DVE compares
           (first-max tie-break identical to np.argmax).
  phase B: gather-free tree cascade: child(g, m) = 4g+1+m, so
           sel[d+1][4j+f] = sel[d][j] * (dec[j] == f), levels written into a
           [5632, tok] mask via partition-interleave SBUF DMAs.
  phase C: masked = act * mask (DVE, out bf16), out.T [1024d, 512tok]
           accumulated over all 176 h-tiles in all 8 PSUM banks, bf16 matmuls.
"""
import sys

if "/opt/trn_rl_repo" not in sys.path:
    sys.path.insert(0, "/opt/trn_rl_repo")

from contextlib import ExitStack

import numpy as np
import ml_dtypes

import concourse.bass as bass  # noqa: F401
import concourse.mybir as mybir
import concourse.tile as tile
from concourse import bacc
from concourse.bass_utils import run_bass_kernel_spmd

F32 = mybir.dt.float32
F32R = mybir.dt.float32r
BF16 = mybir.dt.bfloat16

D = 1024
P = 4
DEPTH = 5
FAN = 4
G = 1365
HID = 21840
Gp = 1408            # 11 * 128
Sp = P * Gp          # 5632  (one f-plane)
HIDp = FAN * Sp      # 22528
NT = HIDp // 128     # 176 h-tiles
NPG = Sp // 128      # 44 (p,g) tiles per f-plane
T = 512              # tokens per core
NCORES = 8
KC = D // 128        # 8 contraction chunks

ROUTE_SUBS = 3                  # g < 341 lives in subtiles 0..2 of each p
ROUTE_G = 341                   # groups 0..340 are tree levels 0..4


def tid(p, sub, f):
    return f * NPG + p * 11 + sub


# routing h-tiles ordered (p, sub, f): the four f-planes of one (p,g)-tile are
# adjacent because the group argmax consumes all four together
ROUTE_TILES = [tid(p, sub, f)
               for p in range(P) for sub in range(ROUTE_SUBS) for f in range(FAN)]
ROUTE_SET = set(ROUTE_TILES)
NONROUTE_TILES = [t for t in range(NT) if t not in ROUTE_SET]

# precision split inside the route region
R0_TILES = [tid(p, 0, f) for p in range(P) for f in range(FAN)]    # bf16x3
R12_TILES = [tid(p, s, f) for p in range(P) for s in (1, 2) for f in range(FAN)]
R0_RANK = {t: i for i, t in enumerate(R0_TILES)}
R12_RANK = {t: i for i, t in enumerate(R12_TILES)}

# phase A / phase C production+consumption order
TILE_ORDER = ROUTE_TILES + NONROUTE_TILES


def _adj_groups(order, maxlen=2):
    """Group runs of memory-adjacent entries (up to maxlen) for batched DMA."""
    groups = []
    i = 0
    while i < len(order):
        j = i + 1
        while (j < len(order) and j - i < maxlen
               and order[j] == order[j - 1] + 1):
            j += 1
        groups.append(tuple(order[i:j]))
        i = j
    return groups


NR_GROUPS = _adj_groups(NONROUTE_TILES)
C_GROUPS = _adj_groups(TILE_ORDER)


def _platform(d):
    return (FAN ** d - 1) // 3


def _segments(q0, q1, *bases):
    """Split [q0, q1) at every multiple of 128 relative to each base offset."""
    cuts = {q0, q1}
    for b in bases:
        k = b + ((q0 - b) // 128 + 1) * 128
        while k < q1:
            cuts.add(k)
            k += 128
    cs = sorted(cuts)
    return list(zip(cs[:-1], cs[1:]))


def build_nc():
    nc = bacc.Bacc("TRN2", target_bir_lowering=False, debug=False,
                   num_devices=NCORES)

    xr = nc.dram_tensor("xr", [D, T], F32R, kind="ExternalInput").ap()
    xh = nc.dram_tensor("xh", [D, T], BF16, kind="ExternalInput").ap()
    xl = nc.dram_tensor("xl", [D, T], BF16, kind="ExternalInput").ap()
    Wr0h = nc.dram_tensor("Wr0h", [D, len(R0_TILES) * 128], BF16,
                          kind="ExternalInput").ap()
    Wr0l = nc.dram_tensor("Wr0l", [D, len(R0_TILES) * 128], BF16,
                          kind="ExternalInput").ap()
    Wr12 = nc.dram_tensor("Wr12", [D, len(R12_TILES) * 128], F32R,
                          kind="ExternalInput").ap()
    Wn = nc.dram_tensor("Wn", [D, HIDp], BF16, kind="ExternalInput").ap()
    bvec = nc.dram_tensor("bvec", [128, NT], F32, kind="ExternalInput").ap()
    WoT = nc.dram_tensor("WoT", [HIDp, D], BF16, kind="ExternalInput").ap()
    outT = nc.dram_tensor("outT", [D, T], F32, kind="ExternalOutput").ap()

    with tile.TileContext(nc) as tc, ExitStack() as top:
        perm = top.enter_context(tc.tile_pool(name="perm", bufs=1))
        dramp = top.enter_context(tc.tile_pool(name="dram", bufs=1, space="DRAM"))

        mask = perm.tile([128, NPG, T], BF16)        # 5.8 MB
        dec = perm.tile([128, P * ROUTE_SUBS, T], F32)   # 3.1 MB
        bt = perm.tile([128, NT], F32)
        nc.gpsimd.dma_start(bt[:], bvec[:])
        nc.gpsimd.memset(mask[:], 0.0)

        act_scr = [dramp.tile([128, T], F32 if t in ROUTE_SET else BF16,
                              tag=f"as{t}", name=f"as{t}")
                   for t in range(NT)]

        # ---------------- phase A: matmul1 + gelu + dec ----------------
        with ExitStack() as pa:
            constp = pa.enter_context(tc.tile_pool(name="xconst", bufs=1))
            wr0p = pa.enter_context(tc.tile_pool(name="wr0", bufs=3))
            wr12p = pa.enter_context(tc.tile_pool(name="wr12", bufs=3))
            wfp = pa.enter_context(tc.tile_pool(name="wfull", bufs=5))
            actp = pa.enter_context(tc.tile_pool(name="act", bufs=2))
            tmpp = pa.enter_context(tc.tile_pool(name="tmp", bufs=2))
            psA = pa.enter_context(tc.tile_pool(name="psA", bufs=4, space="PSUM"))

            xtr = constp.tile([128, KC, T], F32R)
            nc.sync.dma_start(xtr[:], xr.rearrange("(c p) t -> p c t", p=128))
            xth = constp.tile([128, KC, T], BF16)
            nc.sync.dma_start(xth[:], xh.rearrange("(c p) t -> p c t", p=128))
            xtl = constp.tile([128, KC, T], BF16)
            nc.sync.dma_start(xtl[:], xl.rearrange("(c p) t -> p c t", p=128))

            route_acts = {}

            def finish_tile(t, ps, adt, tag):
                a = actp.tile([128, T], adt, tag=tag, name=f"a_{t}")
                nc.scalar.activation(a[:], ps[:],
                                     mybir.ActivationFunctionType.Gelu,
                                     bias=bt[:, t:t + 1], scale=1.0)
                nc.sync.dma_start(act_scr[t][:], a[:])
                return a

            for t in ROUTE_TILES:
                f, rem = divmod(t, NPG)
                p, sub = divmod(rem, 11)
                ps = psA.tile([128, T], F32, tag="ps", name=f"ps_{t}")
                if sub == 0:
                    r = R0_RANK[t]
                    w = wr0p.tile([128, KC, 2, 128], BF16, tag="w0")
                    nc.sync.dma_start(
                        w[:, :, 0, :],
                        Wr0h[:, r * 128:(r + 1) * 128]
                        .rearrange("(c p) h -> p c h", p=128))
                    nc.sync.dma_start(
                        w[:, :, 1, :],
                        Wr0l[:, r * 128:(r + 1) * 128]
                        .rearrange("(c p) h -> p c h", p=128))
                    for c in range(KC):
                        nc.tensor.matmul(ps[:], w[:, c, 0, :], xth[:, c, :],
                                         start=(c == 0), stop=False)
                    for c in range(KC):
                        nc.tensor.matmul(ps[:], w[:, c, 0, :], xtl[:, c, :],
                                         start=False, stop=False)
                    for c in range(KC):
                        nc.tensor.matmul(ps[:], w[:, c, 1, :], xth[:, c, :],
                                         start=False, stop=(c == KC - 1))
                else:
                    r = R12_RANK[t]
                    w = wr12p.tile([128, KC, 128], F32R, tag="w12")
                    nc.sync.dma_start(
                        w[:],
                        Wr12[:, r * 128:(r + 1) * 128]
                        .rearrange("(c p) h -> p c h", p=128))
                    for c in range(KC):
                        nc.tensor.matmul(ps[:], w[:, c, :], xtr[:, c, :],
                                         start=(c == 0), stop=(c == KC - 1))
                a = finish_tile(t, ps, F32, f"act{f}")
                route_acts[t] = a
                if f == FAN - 1:
                    # all four f-planes of (p, sub) live -> dec
                    a0 = route_acts.pop(0 * NPG + rem)
                    a1 = route_acts.pop(1 * NPG + rem)
                    a2 = route_acts.pop(2 * NPG + rem)
                    a3 = route_acts.pop(3 * NPG + rem)
                    m01 = tmpp.tile([128, T], F32, tag="m01")
                    m23 = tmpp.tile([128, T], F32, tag="m23")
                    nc.vector.tensor_tensor(m01[:], a0[:], a1[:], mybir.AluOpType.max)
                    nc.vector.tensor_tensor(m23[:], a2[:], a3[:], mybir.AluOpType.max)
                    b1t = tmpp.tile([128, T], mybir.dt.uint8, tag="b1")
                    nc.vector.tensor_tensor(b1t[:], m23[:], m01[:], mybir.AluOpType.is_gt)
                    b01 = tmpp.tile([128, T], F32, tag="b01")
                    nc.vector.tensor_tensor(b01[:], a1[:], a0[:], mybir.AluOpType.is_gt)
                    b23 = tmpp.tile([128, T], F32, tag="b23")
                    nc.vector.tensor_tensor(b23[:], a3[:], a2[:], mybir.AluOpType.is_gt)
                    b0t = tmpp.tile([128, T], F32, tag="b0")
                    nc.vector.select(b0t[:], b1t[:], b23[:], b01[:])
                    nc.vector.scalar_tensor_tensor(
                        dec[:, p * ROUTE_SUBS + sub, :], b1t[:], 2.0, b0t[:],
                        op0=mybir.AluOpType.mult, op1=mybir.AluOpType.add)

            # non-routing tiles, W loaded two memory-adjacent tiles per DMA
            for g in NR_GROUPS:
                t0g = g[0]
                w = wfp.tile([128, KC, 128 * len(g)], BF16, tag="wf")
                nc.sync.dma_start(
                    w[:],
                    Wn[:, t0g * 128:(t0g + len(g)) * 128]
                    .rearrange("(c p) h -> p c h", p=128))
                for u, t in enumerate(g):
                    ps = psA.tile([128, T], F32, tag="ps", name=f"ps_{t}")
                    for c in range(KC):
                        nc.tensor.matmul(ps[:], w[:, c, u * 128:(u + 1) * 128],
                                         xth[:, c, :],
                                         start=(c == 0), stop=(c == KC - 1))
                    finish_tile(t, ps, BF16, "actn")

            # ---------------- phase B: cascade ----------------
            # Engine APs need 32-aligned partition starts: product ops run on
            # 32-aligned padded windows (junk lanes never scattered out).
            # prodI rows are q-aligned (same partition as dec/mask source).
            cascp = pa.enter_context(tc.tile_pool(name="casc", bufs=1))
            prodI = cascp.tile([128, ROUTE_SUBS, FAN, T], BF16)

            for p in range(P):
                base = p * Gp
                # level 0: mask[q=base]=1; rows 1..31 get junk 1.0s that every
                # level-d scatter fully overwrites before level d+1 reads them
                nc.vector.memset(mask[0:32, p * 11, :], 1.0)
                for d in range(DEPTH):
                    plat, platn, n = _platform(d), _platform(d + 1), FAN ** d
                    q0 = base + plat
                    # products (dec[q] == f) * sel[q] -> prodI[q, f]
                    for (qa, qb) in _segments(q0, q0 + n, 0):
                        sub = (qa - base) // 128
                        pr_a, pr_b = qa % 128, (qb - 1) % 128 + 1
                        al_a = pr_a - pr_a % 32
                        al_b = min(128, -(-pr_b // 32) * 32)
                        for f in range(FAN):
                            nc.vector.scalar_tensor_tensor(
                                prodI[al_a:al_b, sub, f, :],
                                dec[al_a:al_b, p * ROUTE_SUBS + sub, :],
                                float(f),
                                mask[al_a:al_b, p * 11 + sub, :],
                                op0=mybir.AluOpType.is_equal,
                                op1=mybir.AluOpType.mult)
                    # scatter prodI -> mask at q' = base + platn + 4j + f
                    c0 = base + platn
                    for r in range(c0 // 128, (c0 + 4 * n - 1) // 128 + 1):
                        lo, hi = max(c0, 128 * r), min(c0 + 4 * n, 128 * (r + 1))
                        for f in range(FAN):
                            ja = max(0, -(-(lo - c0 - f) // 4))
                            jb = min(n, (hi - 1 - c0 - f) // 4 + 1)
                            if ja >= jb:
                                continue
                            for (j1, j2) in _segments(ja, jb, -q0):
                                qsrc = q0 + j1
                                sub = (qsrc - base) // 128
                                pd = (c0 + 4 * j1 + f) % 128
                                nc.sync.dma_start(
                                    mask[pd:pd + 4 * (j2 - j1 - 1) + 1:4, r, :],
                                    prodI[qsrc % 128:qsrc % 128 + (j2 - j1),
                                          sub, f, :])

        # ---------------- phase C: mask-mult + matmul2 ----------------
        with ExitStack() as pc:
            wop = pc.enter_context(tc.tile_pool(name="wo", bufs=4))
            actc = pc.enter_context(tc.tile_pool(name="actc", bufs=6))
            mskp = pc.enter_context(tc.tile_pool(name="msk", bufs=6))
            outp = pc.enter_context(tc.tile_pool(name="outp", bufs=1))
            psC = pc.enter_context(tc.tile_pool(name="psC", bufs=1, space="PSUM"))

            cps = psC.tile([128, KC, T], F32)
            i = 0
            for g in C_GROUPS:
                t0g = g[0]
                wo = wop.tile([128, len(g), D], BF16, tag="wo")
                nc.sync.dma_start(
                    wo[:],
                    WoT[t0g * 128:(t0g + len(g)) * 128, :]
                    .rearrange("(u p) d -> p u d", p=128))
                for u, t in enumerate(g):
                    a = actc.tile([128, T], F32 if t in ROUTE_SET else BF16,
                                  tag="ar" if t in ROUTE_SET else "an")
                    nc.sync.dma_start(a[:], act_scr[t][:])
                    m = mskp.tile([128, T], BF16, tag="m")
                    nc.vector.tensor_tensor(m[:], a[:], mask[:, t % NPG, :],
                                            mybir.AluOpType.mult)
                    for dd in range(KC):
                        nc.tensor.matmul(cps[:, dd, :],
                                         wo[:, u, dd * 128:(dd + 1) * 128], m[:],
                                         start=(i == 0), stop=(i == NT - 1))
                    i += 1

            osb = outp.tile([128, KC, T], F32)
            for dd in range(KC):
                nc.vector.tensor_copy(osb[:, dd, :], cps[:, dd, :])
            nc.sync.dma_start(outT.rearrange("(c p) t -> p c t", p=128), osb[:])

    nc.compile()
    return nc


_NC_CACHE = None


def _get_nc():
    global _NC_CACHE
    if _NC_CACHE is None:
        _NC_CACHE = build_nc()
    return _NC_CACHE


def _prep_inputs(oldx, W_in, b_in, W_out):
    x = np.ascontiguousarray(np.asarray(oldx, np.float32).reshape(-1, D))
    xT = np.ascontiguousarray(x.T)                      # [D, B]
    xTh = xT.astype(ml_dtypes.bfloat16)
    xTl = (xT - xTh.astype(np.float32)).astype(ml_dtypes.bfloat16)

    Wr = np.asarray(W_in, np.float32).reshape(P, G, FAN, D)
    W_dev = np.zeros((FAN, P, Gp, D), np.float32)
    W_dev[:, :, :G, :] = Wr.transpose(2, 0, 1, 3)
    W_dev = W_dev.reshape(HIDp, D)
    WT_dev = np.ascontiguousarray(W_dev.T)              # [D, HIDp]
    WT_bf = WT_dev.astype(ml_dtypes.bfloat16)

    def cols(tiles, arr):
        return np.ascontiguousarray(np.concatenate(
            [arr[:, t * 128:(t + 1) * 128] for t in tiles], axis=1))

    Wr0_32 = cols(R0_TILES, WT_dev)
    Wr0h = Wr0_32.astype(ml_dtypes.bfloat16)
    Wr0l = (Wr0_32 - Wr0h.astype(np.float32)).astype(ml_dtypes.bfloat16)
    Wr12 = cols(R12_TILES, WT_dev)

    br = np.asarray(b_in, np.float32).reshape(P, G, FAN)
    b_dev = np.zeros((FAN, P, Gp), np.float32)
    b_dev[:, :, :G] = br.transpose(2, 0, 1)
    b_dev = np.ascontiguousarray(b_dev.reshape(HIDp).reshape(NT, 128).T)

    Wo = np.asarray(W_out, np.float32).reshape(D, P, G, FAN)
    Wo_dev = np.zeros((FAN, P, Gp, D), np.float32)
    Wo_dev[:, :, :G, :] = Wo.transpose(3, 1, 2, 0)
    WoT_dev = np.ascontiguousarray(Wo_dev.reshape(HIDp, D)).astype(
        ml_dtypes.bfloat16)

    return xT, xTh, xTl, Wr0h, Wr0l, Wr12, WT_bf, b_dev, WoT_dev


def run(oldx, W_in, b_in, W_out, trace=False):
    nc = _get_nc()
    (xT, xTh, xTl, Wr0h, Wr0l, Wr12, WT_bf, b_dev, WoT_dev) = _prep_inputs(
        oldx, W_in, b_in, W_out)

    in_maps = []
    for c in range(NCORES):
        sl = slice(c * T, (c + 1) * T)
        in_maps.append({
            "xr": np.ascontiguousarray(xT[:, sl]),
            "xh": np.ascontiguousarray(xTh[:, sl]),
            "xl": np.ascontiguousarray(xTl[:, sl]),
            "Wr0h": Wr0h, "Wr0l": Wr0l, "Wr12": Wr12,
            "Wn": WT_bf, "bvec": b_dev, "WoT": WoT_dev,
        })
    res = run_bass_kernel_spmd(nc, in_maps, list(range(NCORES)), trace=trace)

    outT = np.concatenate([res.results[c]["outT"] for c in range(NCORES)],
                          axis=1)                        # [D, B]
    out = np.ascontiguousarray(outT.T).reshape(np.asarray(oldx).shape)
    return out.astype(np.float32), res


def kernel(oldx, W_in, b_in, W_out):
    out, _ = run(oldx, W_in, b_in, W_out, trace=False)
    return out


# revision 6
# speedup vs baseline: 1.4658x; 1.1224x over previous
"""Trainium2 Bass kernel for nn_FFFFanout (moe_routing tree-MLP).

Contract: kernel(**inputs) takes FULL unsharded numpy inputs
  oldx  [2, 2048, 1024] f32
  W_in  [21840, 1024]   f32
  b_in  [21840]         f32
  W_out [1024, 21840]   f32
returns [2, 2048, 1024] f32.

Strategy: data-parallel over the 4096 flattened tokens -> 512 per core on 8
NeuronCores. Device layout is "f-major": dev_hid(f, p, g) = f*5632 + p*1408 + g
with G padded 1365 -> 1408 so each (p, g)-plane tile aligns to 128 partitions.
This makes the group-of-4 argmax an elementwise max across 4 f-plane tiles,
the tree mask multiply partition-aligned, and both matmuls contraction-friendly
(activations live as [hid, tokens]).

Precision plan (PE cost: fp32=4 cyc/row, fp32r/bf16=1):
  - routing decisions for tree levels 0..3 (groups 0..84, inside subtile 0 of
    each p) must match fp32 argmax exactly -> logits via 3-pass split-bf16
    (x_hi@W_hi + x_lo@W_hi + x_hi@W_lo, ~20 effective mantissa bits).
  - level-4 decisions (groups 85..340, subtiles 1..2 + tail of 0) tolerate the
    rare flip (it only swaps one leaf contribution) -> single-pass fp32r
    (measured ~12.7 bits on HW).
  - all value paths (non-routing acts, masked acts, both weight matrices) in
    bf16: same PE rate as fp32r, half the HBM traffic.

Per core:
  phase A: logits.T tiles [128h, 512tok] (PSUM, K=1024 in 8 chunks per pass),
           gelu+bias on ACT -> act tiles, spill to DRAM (f32 for route tiles,
           bf16 elsewhere). Group argmax index dec in {0..3} from DVE compares
           (first-max tie-break identical to np.argmax).
  phase B: gather-free tree cascade: child(g, m) = 4g+1+m, so
           sel[d+1][4j+f] = sel[d][j] * (dec[j] == f), levels written into a
           [5632, tok] mask via partition-interleave SBUF DMAs.
  phase C: masked = act * mask (DVE, out bf16), out.T [1024d, 512tok]
           accumulated over all 176 h-tiles in all 8 PSUM banks, bf16 matmuls.
"""
import sys

if "/opt/trn_rl_repo" not in sys.path:
    sys.path.insert(0, "/opt/trn_rl_repo")

from contextlib import ExitStack

import numpy as np
import ml_dtypes

import concourse.bass as bass  # noqa: F401
import concourse.mybir as mybir
import concourse.tile as tile
from concourse import bacc
from concourse.bass_utils import run_bass_kernel_spmd

F32 = mybir.dt.float32
F32R = mybir.dt.float32r
BF16 = mybir.dt.bfloat16

D = 1024
P = 4
DEPTH = 5
FAN = 4
G = 1365
HID = 21840
Gp = 1408            # 11 * 128
Sp = P * Gp          # 5632  (one f-plane)
HIDp = FAN * Sp      # 22528
NT = HIDp // 128     # 176 h-tiles
NPG = Sp // 128      # 44 (p,g) tiles per f-plane
T = 512              # tokens per core
NCORES = 8
KC = D // 128        # 8 contraction chunks

ROUTE_SUBS = 3                  # g < 341 lives in subtiles 0..2 of each p
ROUTE_G = 341                   # groups 0..340 are tree levels 0..4


def tid(p, sub, f):
    return f * NPG + p * 11 + sub


# routing h-tiles ordered (p, sub, f): the four f-planes of one (p,g)-tile are
# adjacent because the group argmax consumes all four together
ROUTE_TILES = [tid(p, sub, f)
               for p in range(P) for sub in range(ROUTE_SUBS) for f in range(FAN)]
ROUTE_SET = set(ROUTE_TILES)
NONROUTE_TILES = [t for t in range(NT) if t not in ROUTE_SET]

# precision split inside the route region
R0_TILES = [tid(p, 0, f) for p in range(P) for f in range(FAN)]    # bf16x3
R12_TILES = [tid(p, s, f) for p in range(P) for s in (1, 2) for f in range(FAN)]
R0_RANK = {t: i for i, t in enumerate(R0_TILES)}
R12_RANK = {t: i for i, t in enumerate(R12_TILES)}

# phase A / phase C production+consumption order
TILE_ORDER = ROUTE_TILES + NONROUTE_TILES


def _adj_groups(order, maxlen=2):
    """Group runs of memory-adjacent entries (up to maxlen) for batched DMA."""
    groups = []
    i = 0
    while i < len(order):
        j = i + 1
        while (j < len(order) and j - i < maxlen
               and order[j] == order[j - 1] + 1):
            j += 1
        groups.append(tuple(order[i:j]))
        i = j
    return groups


NR_GROUPS = _adj_groups(NONROUTE_TILES)
C_GROUPS = _adj_groups(TILE_ORDER)


def _platform(d):
    return (FAN ** d - 1) // 3


def _segments(q0, q1, *bases):
    """Split [q0, q1) at every multiple of 128 relative to each base offset."""
    cuts = {q0, q1}
    for b in bases:
        k = b + ((q0 - b) // 128 + 1) * 128
        while k < q1:
            cuts.add(k)
            k += 128
    cs = sorted(cuts)
    return list(zip(cs[:-1], cs[1:]))


def build_nc():
    nc = bacc.Bacc("TRN2", target_bir_lowering=False, debug=False,
                   num_devices=NCORES)

    xr = nc.dram_tensor("xr", [D, T], F32R, kind="ExternalInput").ap()
    xh = nc.dram_tensor("xh", [D, T], BF16, kind="ExternalInput").ap()
    xl = nc.dram_tensor("xl", [D, T], BF16, kind="ExternalInput").ap()
    Wr0h = nc.dram_tensor("Wr0h", [D, len(R0_TILES) * 128], BF16,
                          kind="ExternalInput").ap()
    Wr0l = nc.dram_tensor("Wr0l", [D, len(R0_TILES) * 128], BF16,
                          kind="ExternalInput").ap()
    Wr12 = nc.dram_tensor("Wr12", [D, len(R12_TILES) * 128], F32R,
                          kind="ExternalInput").ap()
    Wn = nc.dram_tensor("Wn", [D, HIDp], BF16, kind="ExternalInput").ap()
    bvec = nc.dram_tensor("bvec", [128, NT], F32, kind="ExternalInput").ap()
    WoT = nc.dram_tensor("WoT", [HIDp, D], BF16, kind="ExternalInput").ap()
    outT = nc.dram_tensor("outT", [D, T], F32, kind="ExternalOutput").ap()

    with tile.TileContext(nc) as tc, ExitStack() as top:
        perm = top.enter_context(tc.tile_pool(name="perm", bufs=1))
        dramp = top.enter_context(tc.tile_pool(name="dram", bufs=1, space="DRAM"))

        mask = perm.tile([128, NPG, T], BF16)        # 5.8 MB
        dec = perm.tile([128, P * ROUTE_SUBS, T], F32)   # 3.1 MB
        bt = perm.tile([128, NT], F32)
        nc.gpsimd.dma_start(bt[:], bvec[:])
        nc.gpsimd.memset(mask[:], 0.0)

        act_scr = [dramp.tile([128, T], F32 if t in ROUTE_SET else BF16,
                              tag=f"as{t}", name=f"as{t}")
                   for t in range(NT)]

        # ---------------- phase A: matmul1 + gelu + dec ----------------
        with ExitStack() as pa:
            constp = pa.enter_context(tc.tile_pool(name="xconst", bufs=1))
            wr0p = pa.enter_context(tc.tile_pool(name="wr0", bufs=4))
            wr12p = pa.enter_context(tc.tile_pool(name="wr12", bufs=4))
            wfp = pa.enter_context(tc.tile_pool(name="wfull", bufs=6))
            actp = pa.enter_context(tc.tile_pool(name="act", bufs=2))
            tmpp = pa.enter_context(tc.tile_pool(name="tmp", bufs=2))
            psA = pa.enter_context(tc.tile_pool(name="psA", bufs=6, space="PSUM"))
            cascp = pa.enter_context(tc.tile_pool(name="casc", bufs=1))
            prodI = cascp.tile([128, ROUTE_SUBS, FAN, T], BF16)

            xtr = constp.tile([128, KC, T], F32R)
            nc.sync.dma_start(xtr[:], xr.rearrange("(c p) t -> p c t", p=128))
            xth = constp.tile([128, KC, T], BF16)
            nc.sync.dma_start(xth[:], xh.rearrange("(c p) t -> p c t", p=128))
            xtl = constp.tile([128, KC, T], BF16)
            nc.sync.dma_start(xtl[:], xl.rearrange("(c p) t -> p c t", p=128))

            route_acts = {}

            def cascade_p(p):
                """Tree cascade for parallel-slot p: dec[p, 0..2] -> mask
                subtiles of p. Runs interleaved with remaining route/nonroute
                matmuls; scatters go on the idle gpsimd DMA queue so they are
                not stuck behind phase-A weight loads on the sync queue."""
                base = p * Gp
                # level 0: mask[q=base]=1; rows 1..31 get junk 1.0s that every
                # level-d scatter fully overwrites before level d+1 reads them
                nc.vector.memset(mask[0:32, p * 11, :], 1.0)
                for d in range(DEPTH):
                    plat, platn, n = _platform(d), _platform(d + 1), FAN ** d
                    q0 = base + plat
                    # products (dec[q] == f) * sel[q] -> prodI[q, f]
                    for (qa, qb) in _segments(q0, q0 + n, 0):
                        sub = (qa - base) // 128
                        pr_a, pr_b = qa % 128, (qb - 1) % 128 + 1
                        al_a = pr_a - pr_a % 32
                        al_b = min(128, -(-pr_b // 32) * 32)
                        for f in range(FAN):
                            nc.vector.scalar_tensor_tensor(
                                prodI[al_a:al_b, sub, f, :],
                                dec[al_a:al_b, p * ROUTE_SUBS + sub, :],
                                float(f),
                                mask[al_a:al_b, p * 11 + sub, :],
                                op0=mybir.AluOpType.is_equal,
                                op1=mybir.AluOpType.mult)
                    # scatter prodI -> mask at q' = base + platn + 4j + f
                    c0 = base + platn
                    for r in range(c0 // 128, (c0 + 4 * n - 1) // 128 + 1):
                        lo, hi = max(c0, 128 * r), min(c0 + 4 * n, 128 * (r + 1))
                        for f in range(FAN):
                            ja = max(0, -(-(lo - c0 - f) // 4))
                            jb = min(n, (hi - 1 - c0 - f) // 4 + 1)
                            if ja >= jb:
                                continue
                            for (j1, j2) in _segments(ja, jb, -q0):
                                qsrc = q0 + j1
                                sub = (qsrc - base) // 128
                                pd = (c0 + 4 * j1 + f) % 128
                                nc.gpsimd.dma_start(
                                    mask[pd:pd + 4 * (j2 - j1 - 1) + 1:4, r, :],
                                    prodI[qsrc % 128:qsrc % 128 + (j2 - j1),
                                          sub, f, :])

            def finish_tile(t, ps, adt, tag):
                a = actp.tile([128, T], adt, tag=tag, name=f"a_{t}")
                nc.scalar.activation(a[:], ps[:],
                                     mybir.ActivationFunctionType.Gelu,
                                     bias=bt[:, t:t + 1], scale=1.0)
                nc.sync.dma_start(act_scr[t][:], a[:])
                return a

            for t in ROUTE_TILES:
                f, rem = divmod(t, NPG)
                p, sub = divmod(rem, 11)
                ps = psA.tile([128, T], F32, tag="ps", name=f"ps_{t}")
                if sub == 0:
                    r = R0_RANK[t]
                    w = wr0p.tile([128, KC, 2, 128], BF16, tag="w0")
                    nc.sync.dma_start(
                        w[:, :, 0, :],
                        Wr0h[:, r * 128:(r + 1) * 128]
                        .rearrange("(c p) h -> p c h", p=128))
                    nc.sync.dma_start(
                        w[:, :, 1, :],
                        Wr0l[:, r * 128:(r + 1) * 128]
                        .rearrange("(c p) h -> p c h", p=128))
                    for c in range(KC):
                        nc.tensor.matmul(ps[:], w[:, c, 0, :], xth[:, c, :],
                                         start=(c == 0), stop=False)
                    for c in range(KC):
                        nc.tensor.matmul(ps[:], w[:, c, 0, :], xtl[:, c, :],
                                         start=False, stop=False)
                    for c in range(KC):
                        nc.tensor.matmul(ps[:], w[:, c, 1, :], xth[:, c, :],
                                         start=False, stop=(c == KC - 1))
                else:
                    r = R12_RANK[t]
                    w = wr12p.tile([128, KC, 128], F32R, tag="w12")
                    nc.sync.dma_start(
                        w[:],
                        Wr12[:, r * 128:(r + 1) * 128]
                        .rearrange("(c p) h -> p c h", p=128))
                    for c in range(KC):
                        nc.tensor.matmul(ps[:], w[:, c, :], xtr[:, c, :],
                                         start=(c == 0), stop=(c == KC - 1))
                a = finish_tile(t, ps, F32, f"act{f}")
                route_acts[t] = a
                if f == FAN - 1:
                    # all four f-planes of (p, sub) live -> dec
                    a0 = route_acts.pop(0 * NPG + rem)
                    a1 = route_acts.pop(1 * NPG + rem)
                    a2 = route_acts.pop(2 * NPG + rem)
                    a3 = route_acts.pop(3 * NPG + rem)
                    m01 = tmpp.tile([128, T], F32, tag="m01")
                    m23 = tmpp.tile([128, T], F32, tag="m23")
                    nc.vector.tensor_tensor(m01[:], a0[:], a1[:], mybir.AluOpType.max)
                    nc.vector.tensor_tensor(m23[:], a2[:], a3[:], mybir.AluOpType.max)
                    b1t = tmpp.tile([128, T], mybir.dt.uint8, tag="b1")
                    nc.vector.tensor_tensor(b1t[:], m23[:], m01[:], mybir.AluOpType.is_gt)
                    b01 = tmpp.tile([128, T], F32, tag="b01")
                    nc.vector.tensor_tensor(b01[:], a1[:], a0[:], mybir.AluOpType.is_gt)
                    b23 = tmpp.tile([128, T], F32, tag="b23")
                    nc.vector.tensor_tensor(b23[:], a3[:], a2[:], mybir.AluOpType.is_gt)
                    b0t = tmpp.tile([128, T], F32, tag="b0")
                    nc.vector.select(b0t[:], b1t[:], b23[:], b01[:])
                    nc.vector.scalar_tensor_tensor(
                        dec[:, p * ROUTE_SUBS + sub, :], b1t[:], 2.0, b0t[:],
                        op0=mybir.AluOpType.mult, op1=mybir.AluOpType.add)
                    if sub == ROUTE_SUBS - 1:
                        cascade_p(p)

            # non-routing tiles, W loaded two memory-adjacent tiles per DMA
            for g in NR_GROUPS:
                t0g = g[0]
                w = wfp.tile([128, KC, 128 * len(g)], BF16, tag="wf")
                nc.sync.dma_start(
                    w[:],
                    Wn[:, t0g * 128:(t0g + len(g)) * 128]
                    .rearrange("(c p) h -> p c h", p=128))
                for u, t in enumerate(g):
                    ps = psA.tile([128, T], F32, tag="ps", name=f"ps_{t}")
                    for c in range(KC):
                        nc.tensor.matmul(ps[:], w[:, c, u * 128:(u + 1) * 128],
                                         xth[:, c, :],
                                         start=(c == 0), stop=(c == KC - 1))
                    finish_tile(t, ps, BF16, "actn")

        # ---------------- phase C: mask-mult + matmul2 ----------------
        with ExitStack() as pc:
            wop = pc.enter_context(tc.tile_pool(name="wo", bufs=4))
            actc = pc.enter_context(tc.tile_pool(name="actc", bufs=6))
            mskp = pc.enter_context(tc.tile_pool(name="msk", bufs=6))
            outp = pc.enter_context(tc.tile_pool(name="outp", bufs=1))
            psC = pc.enter_context(tc.tile_pool(name="psC", bufs=1, space="PSUM"))

            cps = psC.tile([128, KC, T], F32)
            i = 0
            for g in C_GROUPS:
                t0g = g[0]
                wo = wop.tile([128, len(g), D], BF16, tag="wo")
                nc.sync.dma_start(
                    wo[:],
                    WoT[t0g * 128:(t0g + len(g)) * 128, :]
                    .rearrange("(u p) d -> p u d", p=128))
                for u, t in enumerate(g):
                    a = actc.tile([128, T], F32 if t in ROUTE_SET else BF16,
                                  tag="ar" if t in ROUTE_SET else "an")
                    nc.sync.dma_start(a[:], act_scr[t][:])
                    m = mskp.tile([128, T], BF16, tag="m")
                    nc.vector.tensor_tensor(m[:], a[:], mask[:, t % NPG, :],
                                            mybir.AluOpType.mult)
                    for dd in range(KC):
                        nc.tensor.matmul(cps[:, dd, :],
                                         wo[:, u, dd * 128:(dd + 1) * 128], m[:],
                                         start=(i == 0), stop=(i == NT - 1))
                    i += 1

            osb = outp.tile([128, KC, T], F32)
            for dd in range(KC):
                nc.vector.tensor_copy(osb[:, dd, :], cps[:, dd, :])
            nc.sync.dma_start(outT.rearrange("(c p) t -> p c t", p=128), osb[:])

    nc.compile()
    return nc


_NC_CACHE = None


def _get_nc():
    global _NC_CACHE
    if _NC_CACHE is None:
        _NC_CACHE = build_nc()
    return _NC_CACHE


def _prep_inputs(oldx, W_in, b_in, W_out):
    x = np.ascontiguousarray(np.asarray(oldx, np.float32).reshape(-1, D))
    xT = np.ascontiguousarray(x.T)                      # [D, B]
    xTh = xT.astype(ml_dtypes.bfloat16)
    xTl = (xT - xTh.astype(np.float32)).astype(ml_dtypes.bfloat16)

    Wr = np.asarray(W_in, np.float32).reshape(P, G, FAN, D)
    W_dev = np.zeros((FAN, P, Gp, D), np.float32)
    W_dev[:, :, :G, :] = Wr.transpose(2, 0, 1, 3)
    W_dev = W_dev.reshape(HIDp, D)
    WT_dev = np.ascontiguousarray(W_dev.T)              # [D, HIDp]
    WT_bf = WT_dev.astype(ml_dtypes.bfloat16)

    def cols(tiles, arr):
        return np.ascontiguousarray(np.concatenate(
            [arr[:, t * 128:(t + 1) * 128] for t in tiles], axis=1))

    Wr0_32 = cols(R0_TILES, WT_dev)
    Wr0h = Wr0_32.astype(ml_dtypes.bfloat16)
    Wr0l = (Wr0_32 - Wr0h.astype(np.float32)).astype(ml_dtypes.bfloat16)
    Wr12 = cols(R12_TILES, WT_dev)

    br = np.asarray(b_in, np.float32).reshape(P, G, FAN)
    b_dev = np.zeros((FAN, P, Gp), np.float32)
    b_dev[:, :, :G] = br.transpose(2, 0, 1)
    b_dev = np.ascontiguousarray(b_dev.reshape(HIDp).reshape(NT, 128).T)

    Wo = np.asarray(W_out, np.float32).reshape(D, P, G, FAN)
    Wo_dev = np.zeros((FAN, P, Gp, D), np.float32)
    Wo_dev[:, :, :G, :] = Wo.transpose(3, 1, 2, 0)
    WoT_dev = np.ascontiguousarray(Wo_dev.reshape(HIDp, D)).astype(
        ml_dtypes.bfloat16)

    return xT, xTh, xTl, Wr0h, Wr0l, Wr12, WT_bf, b_dev, WoT_dev


def run(oldx, W_in, b_in, W_out, trace=False):
    nc = _get_nc()
    (xT, xTh, xTl, Wr0h, Wr0l, Wr12, WT_bf, b_dev, WoT_dev) = _prep_inputs(
        oldx, W_in, b_in, W_out)

    in_maps = []
    for c in range(NCORES):
        sl = slice(c * T, (c + 1) * T)
        in_maps.append({
            "xr": np.ascontiguousarray(xT[:, sl]),
            "xh": np.ascontiguousarray(xTh[:, sl]),
            "xl": np.ascontiguousarray(xTl[:, sl]),
            "Wr0h": Wr0h, "Wr0l": Wr0l, "Wr12": Wr12,
            "Wn": WT_bf, "bvec": b_dev, "WoT": WoT_dev,
        })
    res = run_bass_kernel_spmd(nc, in_maps, list(range(NCORES)), trace=trace)

    outT = np.concatenate([res.results[c]["outT"] for c in range(NCORES)],
                          axis=1)                        # [D, B]
    out = np.ascontiguousarray(outT.T).reshape(np.asarray(oldx).shape)
    return out.astype(np.float32), res


def kernel(oldx, W_in, b_in, W_out):
    out, _ = run(oldx, W_in, b_in, W_out, trace=False)
    return out
